# revision 9
# baseline (speedup 1.0000x reference)
"""ArcFace loss on 8 TRN2 NeuronCores — class-axis (vocab) parallel.

Full inputs in, full scalar loss out. Classes sharded 12500/core; x and the
gathered target weight rows are replicated. Per-core: normalize, bf16 matmul
x_norm @ w_normT, fused exp(s*cos(theta+m)) epilogue with row-sum
accumulation, one 8KB AllReduce of the row sums, then an exact f32
target-term correction + log + mean computed redundantly on every core.
"""

import math

import numpy as np

import concourse.bass as bass
import concourse.tile as tile
from concourse import bacc, masks, mybir
from concourse.bass_utils import run_bass_kernel_spmd

B = 2048
D = 128
C = 100000
NCORES = 8
CS = C // NCORES          # 12500 classes per core
NBT = B // 128            # 16 batch tiles
NWT = (CS + 127) // 128   # 98 class tiles (last one is 84 rows)
WTAIL = CS - (NWT - 1) * 128  # 84
CHUNK = 500               # main-loop free-dim chunk (fits one PSUM bank)
NCK = CS // CHUNK         # 25

MARGIN = 0.5
SCALE = 70.0
COS_M = math.cos(MARGIN)
SIN_M = math.sin(MARGIN)
MM = math.sin(math.pi - MARGIN) * MARGIN
K1 = SCALE * COS_M        # 61.43...
K2 = SCALE * SIN_M        # 33.56...
K2SQ = K2 * K2

F32 = mybir.dt.float32
F16 = mybir.dt.float16
BF16 = mybir.dt.bfloat16
AF = mybir.ActivationFunctionType
ALU = mybir.AluOpType

_NC = None


def _build():
    nc = bacc.Bacc(
        "TRN2", target_bir_lowering=False, debug=False, num_devices=NCORES)
    x_ext = nc.declare_dram_parameter("x", [B, D], F32, isOutput=False)
    w_ext = nc.declare_dram_parameter("w", [CS, D], F32, isOutput=False)
    wtg_ext = nc.declare_dram_parameter("wtg", [B, D], F32, isOutput=False)
    out_ext = nc.declare_dram_parameter("out", [1, 1], F32, isOutput=True)

    with tile.TileContext(nc) as tc:
        from contextlib import ExitStack

        with ExitStack() as ctx:
            singles = ctx.enter_context(tc.tile_pool(name="singles", bufs=1))
            scr = ctx.enter_context(tc.tile_pool(name="scr", bufs=3))
            up = ctx.enter_context(tc.tile_pool(name="up", bufs=3))
            vp = ctx.enter_context(tc.tile_pool(name="vp", bufs=3))
            ap_ = ctx.enter_context(tc.tile_pool(name="ap", bufs=3))
            ep = ctx.enter_context(tc.tile_pool(name="ep", bufs=2))
            mm_pool = ctx.enter_context(
                tc.tile_pool(name="mm", bufs=4, space="PSUM"))
            tp_pool = ctx.enter_context(
                tc.tile_pool(name="tp", bufs=2, space="PSUM"))
            fin_pool = ctx.enter_context(
                tc.tile_pool(name="fin", bufs=1, space="PSUM"))
            dram = ctx.enter_context(
                tc.tile_pool(name="dram", bufs=1, space="DRAM"))

            ident = singles.tile([128, 128], BF16)
            masks.make_identity(nc, ident[:])
            ones = singles.tile([128, 1], F32)
            nc.gpsimd.memset(ones[:], 1.0)
            k2sqb = singles.tile([128, 1], F32)
            nc.gpsimd.memset(k2sqb[:], K2SQ)

            # ---------------- load x / wtg (replicated) ----------------
            x_all = singles.tile([128, B], F32)      # col bt*128+d
            nc.sync.dma_start(
                out=x_all[:].rearrange("p (a d) -> p a d", d=D),
                in_=x_ext.rearrange("(a p) d -> p a d", p=128))
            wtg_all = singles.tile([128, B], F32)
            nc.sync.dma_start(
                out=wtg_all[:].rearrange("p (a d) -> p a d", d=D),
                in_=wtg_ext.rearrange("(a p) d -> p a d", p=128))

            # ---------------- load w shard ----------------
            NFULL = NWT - 1  # 97 full 128-row tiles
            w_all = singles.tile([128, NFULL * 128], F32)
            rows_per_dma = 12 * 128
            ndma = (NFULL * 128) // rows_per_dma  # 8 DMAs of 1536 rows
            for i in range(ndma):
                r0 = i * rows_per_dma
                nc.sync.dma_start(
                    out=w_all[:, r0:r0 + rows_per_dma].rearrange(
                        "p (a d) -> p a d", d=D),
                    in_=w_ext[r0:r0 + rows_per_dma, :].rearrange(
                        "(a p) d -> p a d", p=128))
            rem0 = ndma * rows_per_dma
            if rem0 < NFULL * 128:
                nc.sync.dma_start(
                    out=w_all[:, rem0:NFULL * 128].rearrange(
                        "p (a d) -> p a d", d=D),
                    in_=w_ext[rem0:NFULL * 128, :].rearrange(
                        "(a p) d -> p a d", p=128))
            w_tail = singles.tile([128, 128], F32)
            nc.sync.dma_start(
                out=w_tail[:WTAIL, :], in_=w_ext[NFULL * 128:CS, :])

            # ---------------- normalize w, build wnT [128, CS] bf16 ------
            ns2 = singles.tile([128, NWT], F32)
            for t in range(NWT):
                if t < NFULL:
                    wt = w_all[:, t * 128:(t + 1) * 128]
                    psz = 128
                else:
                    psz = WTAIL
                    wt = w_tail[:psz, :]
                sq = scr.tile([128, 128], F32, tag="sq")
                nc.scalar.activation(
                    sq[:psz, :], wt, AF.Square,
                    accum_out=ns2[:psz, t:t + 1])
            wnrm = singles.tile([128, NWT], F32)
            nc.scalar.activation(wnrm[:], ns2[:], AF.Sqrt)
            winv = singles.tile([128, NWT], F32)
            nc.vector.reciprocal(winv[:], wnrm[:])

            wnT = singles.tile([128, CS], BF16)
            for t in range(NWT):
                if t < NFULL:
                    wt = w_all[:, t * 128:(t + 1) * 128]
                    psz = 128
                else:
                    psz = WTAIL
                    wt = w_tail[:psz, :]
                wn = scr.tile([128, 128], BF16, tag="wn")
                nc.vector.tensor_scalar(
                    out=wn[:psz, :], in0=wt, scalar1=winv[:psz, t:t + 1],
                    scalar2=None, op0=ALU.mult)
                tp = tp_pool.tile([128, 128], BF16)
                nc.tensor.transpose(tp[:, :psz], wn[:psz, :], ident[:psz, :psz])
                nc.scalar.activation(
                    wnT[:, t * 128:t * 128 + psz], tp[:, :psz], AF.Copy)

            # ---------------- normalize x, build xnT [128, B] bf16 -------
            xs2 = singles.tile([128, NBT], F32)
            for t in range(NBT):
                xt = x_all[:, t * 128:(t + 1) * 128]
                sq = scr.tile([128, 128], F32, tag="sq")
                nc.scalar.activation(
                    sq[:], xt, AF.Square, accum_out=xs2[:, t:t + 1])
            xnrm = singles.tile([128, NBT], F32)
            nc.scalar.activation(xnrm[:], xs2[:], AF.Sqrt)
            xinv = singles.tile([128, NBT], F32)
            nc.vector.reciprocal(xinv[:], xnrm[:])

            xnT = singles.tile([128, B], BF16)
            for t in range(NBT):
                xt = x_all[:, t * 128:(t + 1) * 128]
                xn = scr.tile([128, 128], BF16, tag="wn")
                nc.vector.tensor_scalar(
                    out=xn[:], in0=xt, scalar1=xinv[:, t:t + 1],
                    scalar2=None, op0=ALU.mult)
                tp = tp_pool.tile([128, 128], BF16)
                nc.tensor.transpose(tp[:], xn[:], ident[:])
                nc.scalar.activation(
                    xnT[:, t * 128:(t + 1) * 128], tp[:], AF.Copy)

            # ---------------- main loop: cos -> exp -> row partial sums --
            sums = singles.tile([128, NBT * NCK], F32)
            for bt in range(NBT):
                lhsT = xnT[:, bt * 128:(bt + 1) * 128]
                for ck in range(NCK):
                    pc = mm_pool.tile([128, CHUNK], F32)
                    nc.tensor.matmul(
                        pc[:], lhsT, wnT[:, ck * CHUNK:(ck + 1) * CHUNK],
                        start=True, stop=True)
                    c = up.tile([128, CHUNK], F16, tag="c")
                    nc.vector.tensor_copy(c[:], pc[:])
                    u = up.tile([128, CHUNK], F16)
                    nc.vector.scalar_tensor_tensor(
                        out=u[:], in0=c[:], scalar=1.0, in1=c[:],
                        op0=ALU.mult, op1=ALU.mult)
                    v = vp.tile([128, CHUNK], F16)
                    nc.scalar.activation(
                        v[:], u[:], AF.Sqrt, scale=-K2SQ, bias=k2sqb[:])
                    a = ap_.tile([128, CHUNK], F16)
                    nc.vector.scalar_tensor_tensor(
                        out=a[:], in0=c[:], scalar=K1, in1=v[:],
                        op0=ALU.mult, op1=ALU.subtract)
                    e = ep.tile([128, CHUNK], BF16)
                    col = bt * NCK + ck
                    nc.scalar.activation(
                        e[:], a[:], AF.Exp,
                        accum_out=sums[:, col:col + 1])

            rs = singles.tile([128, NBT], F32)
            nc.vector.tensor_reduce(
                rs[:], sums[:].rearrange("p (a b) -> p a b", b=NCK),
                axis=mybir.AxisListType.X, op=ALU.add)

            # ---------------- AllReduce row sums (8KB) -------------------
            rs_in = dram.tile([128, NBT], F32)
            rs_out = dram.tile([128, NBT], F32)
            nc.sync.dma_start(rs_in[:], rs[:])
            nc.gpsimd.collective_compute(
                "AllReduce", ALU.add,
                replica_groups=[list(range(NCORES))],
                ins=[rs_in.opt()], outs=[rs_out.opt()])
            rsum = singles.tile([128, NBT], F32)
            nc.sync.dma_start(rsum[:], rs_out[:])

            # ---------------- exact target-term correction (f32) ---------
            ws2 = singles.tile([128, NBT], F32)
            dots = singles.tile([128, NBT], F32)
            for t in range(NBT):
                gt = wtg_all[:, t * 128:(t + 1) * 128]
                xt = x_all[:, t * 128:(t + 1) * 128]
                sq = scr.tile([128, 128], F32, tag="sq")
                nc.scalar.activation(
                    sq[:], gt, AF.Square, accum_out=ws2[:, t:t + 1])
                dt_ = scr.tile([128, 128], F32, tag="sq")
                nc.vector.tensor_tensor(dt_[:], gt, xt, op=ALU.mult)
                nc.vector.tensor_reduce(
                    dots[:, t:t + 1], dt_[:], axis=mybir.AxisListType.XYZW,
                    op=ALU.add)
            wgn = singles.tile([128, NBT], F32)
            nc.scalar.activation(wgn[:], ws2[:], AF.Sqrt)
            wgi = singles.tile([128, NBT], F32)
            nc.vector.reciprocal(wgi[:], wgn[:])

            ct0 = singles.tile([128, NBT], F32)
            nc.vector.tensor_tensor(ct0[:], dots[:], xinv[:], op=ALU.mult)
            ct = singles.tile([128, NBT], F32)
            nc.vector.tensor_tensor(ct[:], ct0[:], wgi[:], op=ALU.mult)

            u2 = singles.tile([128, NBT], F32)
            nc.vector.tensor_tensor(u2[:], ct[:], ct[:], op=ALU.mult)
            v2 = singles.tile([128, NBT], F32)
            nc.scalar.activation(v2[:], u2[:], AF.Sqrt, scale=-K2SQ,
                                 bias=k2sqb[:])
            a2 = singles.tile([128, NBT], F32)
            nc.vector.scalar_tensor_tensor(
                out=a2[:], in0=ct[:], scalar=K1, in1=v2[:],
                op0=ALU.mult, op1=ALU.subtract)
            t1 = singles.tile([128, NBT], F32)
            nc.scalar.activation(t1[:], a2[:], AF.Exp)
            a3 = singles.tile([128, NBT], F32)
            nc.vector.tensor_scalar(
                out=a3[:], in0=ct[:], scalar1=SCALE, scalar2=-SCALE * MM,
                op0=ALU.mult, op1=ALU.add)
            t2 = singles.tile([128, NBT], F32)
            nc.scalar.activation(t2[:], a3[:], AF.Exp)

            s0 = singles.tile([128, NBT], F32)
            nc.vector.tensor_tensor(s0[:], rsum[:], t1[:], op=ALU.subtract)
            s1 = singles.tile([128, NBT], F32)
            nc.vector.tensor_tensor(s1[:], s0[:], t2[:], op=ALU.add)
            lse = singles.tile([128, NBT], F32)
            nc.scalar.activation(lse[:], s1[:], AF.Ln)
            loss = singles.tile([128, NBT], F32)
            nc.vector.tensor_tensor(loss[:], lse[:], a3[:], op=ALU.subtract)

            lscr = singles.tile([128, NBT], F32)
            lcol = singles.tile([128, 1], F32)
            nc.scalar.activation(
                lscr[:], loss[:], AF.Identity, scale=1.0 / B,
                accum_out=lcol[:])
            fin = fin_pool.tile([1, 1], F32)
            nc.tensor.matmul(fin[:1, :1], ones[:], lcol[:],
                             start=True, stop=True)
            out_sb = singles.tile([1, 1], F32)
            nc.scalar.activation(out_sb[:1, :1], fin[:1, :1], AF.Copy)
            nc.sync.dma_start(out_ext[:, :], out_sb[:1, :1])

    nc.finalize()
    return nc


def _get_nc():
    global _NC
    if _NC is None:
        _NC = _build()
    return _NC


def _in_maps(inputs):
    x = np.ascontiguousarray(np.asarray(inputs["x"], dtype=np.float32))
    target = np.asarray(inputs["target"]).astype(np.int64)
    weight = np.ascontiguousarray(
        np.asarray(inputs["weight"], dtype=np.float32))
    wtg = np.ascontiguousarray(weight[target])
    maps = []
    for c in range(NCORES):
        shard = np.ascontiguousarray(weight[c * CS:(c + 1) * CS])
        maps.append({"x": x, "w": shard, "wtg": wtg})
    return maps


def run(inputs, trace=False, **kw):
    res = run_bass_kernel_spmd(
        _get_nc(), _in_maps(inputs), core_ids=list(range(NCORES)),
        trace=trace, **kw)
    out = np.asarray(res.results[0]["out"], dtype=np.float32).reshape(())
    return out, res


def kernel(**inputs):
    out, _ = run(inputs, trace=False)
    return out


# revision 10
# speedup vs baseline: 1.3529x; 1.3529x over previous
"""ArcFace loss on 8 TRN2 NeuronCores — class-axis (vocab) parallel.

Full inputs in, full scalar loss out. Classes sharded 12500/core; x and the
gathered target weight rows are replicated. Per-core: normalize, bf16 matmul
x_norm @ w_normT, fused exp(s*cos(theta+m)) epilogue with row-sum
accumulation, one 8KB AllReduce of the row sums, then an exact f32
target-term correction + log + mean computed redundantly on every core.
"""

import math

import numpy as np

import concourse.bass as bass
import concourse.tile as tile
from concourse import bacc, masks, mybir
from concourse.bass_utils import run_bass_kernel_spmd

B = 2048
D = 128
C = 100000
NCORES = 8
CS = C // NCORES          # 12500 classes per core
NBT = B // 128            # 16 batch tiles
NWT = (CS + 127) // 128   # 98 class tiles (last one is 84 rows)
WTAIL = CS - (NWT - 1) * 128  # 84
CHUNK = 500               # main-loop free-dim chunk (fits one PSUM bank)
NCK = CS // CHUNK         # 25

MARGIN = 0.5
SCALE = 70.0
COS_M = math.cos(MARGIN)
SIN_M = math.sin(MARGIN)
MM = math.sin(math.pi - MARGIN) * MARGIN
K1 = SCALE * COS_M        # 61.43...
K2 = SCALE * SIN_M        # 33.56...
K2SQ = K2 * K2

F32 = mybir.dt.float32
F16 = mybir.dt.float16
BF16 = mybir.dt.bfloat16
AF = mybir.ActivationFunctionType
ALU = mybir.AluOpType

_NC = None


def _build():
    nc = bacc.Bacc(
        "TRN2", target_bir_lowering=False, debug=False, num_devices=NCORES)
    x_ext = nc.declare_dram_parameter("x", [B, D], F32, isOutput=False)
    w_ext = nc.declare_dram_parameter("w", [CS, D], F32, isOutput=False)
    wtg_ext = nc.declare_dram_parameter("wtg", [B, D], F32, isOutput=False)
    out_ext = nc.declare_dram_parameter("out", [1, 1], F32, isOutput=True)

    with tile.TileContext(nc) as tc:
        from contextlib import ExitStack

        with ExitStack() as ctx:
            singles = ctx.enter_context(tc.tile_pool(name="singles", bufs=1))
            scr = ctx.enter_context(tc.tile_pool(name="scr", bufs=3))
            mm_pool = ctx.enter_context(
                tc.tile_pool(name="mm", bufs=4, space="PSUM"))
            tp_pool = ctx.enter_context(
                tc.tile_pool(name="tp", bufs=2, space="PSUM"))
            fin_pool = ctx.enter_context(
                tc.tile_pool(name="fin", bufs=1, space="PSUM"))
            dram = ctx.enter_context(
                tc.tile_pool(name="dram", bufs=1, space="DRAM"))

            ident = singles.tile([128, 128], BF16)
            masks.make_identity(nc, ident[:])
            ones = singles.tile([128, 1], F32)
            nc.gpsimd.memset(ones[:], 1.0)
            k2sqb = singles.tile([128, 1], F32)
            nc.gpsimd.memset(k2sqb[:], K2SQ)

            # ---------------- load x / wtg (replicated) ----------------
            x_all = singles.tile([128, B], F32)      # col bt*128+d
            nc.sync.dma_start(
                out=x_all[:].rearrange("p (a d) -> p a d", d=D),
                in_=x_ext.rearrange("(a p) d -> p a d", p=128))
            wtg_all = singles.tile([128, B], F32)
            nc.sync.dma_start(
                out=wtg_all[:].rearrange("p (a d) -> p a d", d=D),
                in_=wtg_ext.rearrange("(a p) d -> p a d", p=128))

            # ---------------- load w shard (scoped pool, freed later) ----
            NFULL = NWT - 1  # 97 full 128-row tiles
            wload_ctx = ExitStack()
            wload = wload_ctx.enter_context(
                tc.tile_pool(name="wload", bufs=1))
            w_all = wload.tile([128, NFULL * 128], F32)
            rows_per_dma = 12 * 128
            ndma = (NFULL * 128) // rows_per_dma  # 8 DMAs of 1536 rows
            for i in range(ndma):
                r0 = i * rows_per_dma
                nc.sync.dma_start(
                    out=w_all[:, r0:r0 + rows_per_dma].rearrange(
                        "p (a d) -> p a d", d=D),
                    in_=w_ext[r0:r0 + rows_per_dma, :].rearrange(
                        "(a p) d -> p a d", p=128))
            rem0 = ndma * rows_per_dma
            if rem0 < NFULL * 128:
                nc.sync.dma_start(
                    out=w_all[:, rem0:NFULL * 128].rearrange(
                        "p (a d) -> p a d", d=D),
                    in_=w_ext[rem0:NFULL * 128, :].rearrange(
                        "(a p) d -> p a d", p=128))
            w_tail = wload.tile([128, 128], F32)
            nc.sync.dma_start(
                out=w_tail[:WTAIL, :], in_=w_ext[NFULL * 128:CS, :])

            # ---------------- normalize w, build wnT [128, CS] bf16 ------
            ns2 = singles.tile([128, NWT], F32)
            for t in range(NWT):
                if t < NFULL:
                    wt = w_all[:, t * 128:(t + 1) * 128]
                    psz = 128
                else:
                    psz = WTAIL
                    wt = w_tail[:psz, :]
                sq = scr.tile([128, 128], F32, tag="sq")
                nc.scalar.activation(
                    sq[:psz, :], wt, AF.Square,
                    accum_out=ns2[:psz, t:t + 1])
            wnrm = singles.tile([128, NWT], F32)
            nc.scalar.activation(wnrm[:], ns2[:], AF.Sqrt)
            winv = singles.tile([128, NWT], F32)
            nc.vector.reciprocal(winv[:], wnrm[:])

            wnT = singles.tile([128, CS], BF16)
            for t in range(NWT):
                if t < NFULL:
                    wt = w_all[:, t * 128:(t + 1) * 128]
                    psz = 128
                else:
                    psz = WTAIL
                    wt = w_tail[:psz, :]
                wn = scr.tile([128, 128], BF16, tag="wn")
                nc.vector.tensor_scalar(
                    out=wn[:psz, :], in0=wt, scalar1=winv[:psz, t:t + 1],
                    scalar2=None, op0=ALU.mult)
                tp = tp_pool.tile([128, 128], BF16)
                nc.tensor.transpose(tp[:, :psz], wn[:psz, :], ident[:psz, :psz])
                nc.scalar.activation(
                    wnT[:, t * 128:t * 128 + psz], tp[:, :psz], AF.Copy)

            wload_ctx.close()

            # ---------------- normalize x, build xnT [128, B] bf16 -------
            xs2 = singles.tile([128, NBT], F32)
            for t in range(NBT):
                xt = x_all[:, t * 128:(t + 1) * 128]
                sq = scr.tile([128, 128], F32, tag="sq")
                nc.scalar.activation(
                    sq[:], xt, AF.Square, accum_out=xs2[:, t:t + 1])
            xnrm = singles.tile([128, NBT], F32)
            nc.scalar.activation(xnrm[:], xs2[:], AF.Sqrt)
            xinv = singles.tile([128, NBT], F32)
            nc.vector.reciprocal(xinv[:], xnrm[:])

            xnT = singles.tile([128, B], BF16)
            for t in range(NBT):
                xt = x_all[:, t * 128:(t + 1) * 128]
                xn = scr.tile([128, 128], BF16, tag="wn")
                nc.vector.tensor_scalar(
                    out=xn[:], in0=xt, scalar1=xinv[:, t:t + 1],
                    scalar2=None, op0=ALU.mult)
                tp = tp_pool.tile([128, 128], BF16)
                nc.tensor.transpose(tp[:], xn[:], ident[:])
                nc.scalar.activation(
                    xnT[:, t * 128:(t + 1) * 128], tp[:], AF.Copy)

            # ---------------- main loop: cos -> exp -> row partial sums --
            # Phase-batched per half-row-block so ACT runs one wide Sqrt and
            # one wide Exp per group (avoids per-chunk act-table reloads).
            GROUPS = [(0, 13), (13, 25)]
            GW = 13 * CHUNK  # 6500, max group width
            cg = ctx.enter_context(tc.tile_pool(name="cg", bufs=2))
            ug = ctx.enter_context(tc.tile_pool(name="ug", bufs=2))
            vg = ctx.enter_context(tc.tile_pool(name="vg", bufs=2))
            ag = ctx.enter_context(tc.tile_pool(name="ag", bufs=2))
            eg = ctx.enter_context(tc.tile_pool(name="eg", bufs=1))
            rs2 = singles.tile([128, NBT * 2], F32)
            for bt in range(NBT):
                lhsT = xnT[:, bt * 128:(bt + 1) * 128]
                for g, (c0, c1) in enumerate(GROUPS):
                    W = (c1 - c0) * CHUNK
                    c_all = cg.tile([128, GW], F16, tag="c")
                    for ck in range(c0, c1):
                        pc = mm_pool.tile([128, CHUNK], F32)
                        nc.tensor.matmul(
                            pc[:], lhsT, wnT[:, ck * CHUNK:(ck + 1) * CHUNK],
                            start=True, stop=True)
                        o = (ck - c0) * CHUNK
                        nc.vector.tensor_copy(c_all[:, o:o + CHUNK], pc[:])
                    u_all = ug.tile([128, GW], F16, tag="u")
                    nc.vector.scalar_tensor_tensor(
                        out=u_all[:, :W], in0=c_all[:, :W], scalar=1.0,
                        in1=c_all[:, :W], op0=ALU.mult, op1=ALU.mult)
                    v_all = vg.tile([128, GW], F16, tag="v")
                    nc.scalar.activation(
                        v_all[:, :W], u_all[:, :W], AF.Sqrt,
                        scale=-K2SQ, bias=k2sqb[:])
                    a_all = ag.tile([128, GW], F16, tag="a")
                    nc.vector.scalar_tensor_tensor(
                        out=a_all[:, :W], in0=c_all[:, :W], scalar=K1,
                        in1=v_all[:, :W], op0=ALU.mult, op1=ALU.subtract)
                    e_all = eg.tile([128, GW], BF16, tag="e")
                    col = bt * 2 + g
                    nc.scalar.activation(
                        e_all[:, :W], a_all[:, :W], AF.Exp,
                        accum_out=rs2[:, col:col + 1])

            rs = singles.tile([128, NBT], F32)
            rs2v = rs2[:].rearrange("p (a two) -> p a two", two=2)
            nc.vector.tensor_tensor(
                rs[:], rs2v[:, :, 0], rs2v[:, :, 1], op=ALU.add)

            # ---------------- AllReduce row sums (8KB) -------------------
            rs_in = dram.tile([128, NBT], F32)
            rs_out = dram.tile([128, NBT], F32)
            nc.sync.dma_start(rs_in[:], rs[:])
            nc.gpsimd.collective_compute(
                "AllReduce", ALU.add,
                replica_groups=[list(range(NCORES))],
                ins=[rs_in.opt()], outs=[rs_out.opt()])
            rsum = singles.tile([128, NBT], F32)
            nc.sync.dma_start(rsum[:], rs_out[:])

            # ---------------- exact target-term correction (f32) ---------
            ws2 = singles.tile([128, NBT], F32)
            dots = singles.tile([128, NBT], F32)
            for t in range(NBT):
                gt = wtg_all[:, t * 128:(t + 1) * 128]
                xt = x_all[:, t * 128:(t + 1) * 128]
                sq = scr.tile([128, 128], F32, tag="sq")
                nc.scalar.activation(
                    sq[:], gt, AF.Square, accum_out=ws2[:, t:t + 1])
                dt_ = scr.tile([128, 128], F32, tag="sq")
                nc.vector.tensor_tensor(dt_[:], gt, xt, op=ALU.mult)
                nc.vector.tensor_reduce(
                    dots[:, t:t + 1], dt_[:], axis=mybir.AxisListType.XYZW,
                    op=ALU.add)
            wgn = singles.tile([128, NBT], F32)
            nc.scalar.activation(wgn[:], ws2[:], AF.Sqrt)
            wgi = singles.tile([128, NBT], F32)
            nc.vector.reciprocal(wgi[:], wgn[:])

            ct0 = singles.tile([128, NBT], F32)
            nc.vector.tensor_tensor(ct0[:], dots[:], xinv[:], op=ALU.mult)
            ct = singles.tile([128, NBT], F32)
            nc.vector.tensor_tensor(ct[:], ct0[:], wgi[:], op=ALU.mult)

            u2 = singles.tile([128, NBT], F32)
            nc.vector.tensor_tensor(u2[:], ct[:], ct[:], op=ALU.mult)
            v2 = singles.tile([128, NBT], F32)
            nc.scalar.activation(v2[:], u2[:], AF.Sqrt, scale=-K2SQ,
                                 bias=k2sqb[:])
            a2 = singles.tile([128, NBT], F32)
            nc.vector.scalar_tensor_tensor(
                out=a2[:], in0=ct[:], scalar=K1, in1=v2[:],
                op0=ALU.mult, op1=ALU.subtract)
            t1 = singles.tile([128, NBT], F32)
            nc.scalar.activation(t1[:], a2[:], AF.Exp)
            a3 = singles.tile([128, NBT], F32)
            nc.vector.tensor_scalar(
                out=a3[:], in0=ct[:], scalar1=SCALE, scalar2=-SCALE * MM,
                op0=ALU.mult, op1=ALU.add)
            t2 = singles.tile([128, NBT], F32)
            nc.scalar.activation(t2[:], a3[:], AF.Exp)

            s0 = singles.tile([128, NBT], F32)
            nc.vector.tensor_tensor(s0[:], rsum[:], t1[:], op=ALU.subtract)
            s1 = singles.tile([128, NBT], F32)
            nc.vector.tensor_tensor(s1[:], s0[:], t2[:], op=ALU.add)
            lse = singles.tile([128, NBT], F32)
            nc.scalar.activation(lse[:], s1[:], AF.Ln)
            loss = singles.tile([128, NBT], F32)
            nc.vector.tensor_tensor(loss[:], lse[:], a3[:], op=ALU.subtract)

            lscr = singles.tile([128, NBT], F32)
            lcol = singles.tile([128, 1], F32)
            nc.scalar.activation(
                lscr[:], loss[:], AF.Identity, scale=1.0 / B,
                accum_out=lcol[:])
            fin = fin_pool.tile([1, 1], F32)
            nc.tensor.matmul(fin[:1, :1], ones[:], lcol[:],
                             start=True, stop=True)
            out_sb = singles.tile([1, 1], F32)
            nc.scalar.activation(out_sb[:1, :1], fin[:1, :1], AF.Copy)
            nc.sync.dma_start(out_ext[:, :], out_sb[:1, :1])

    nc.finalize()
    return nc


def _get_nc():
    global _NC
    if _NC is None:
        _NC = _build()
    return _NC


def _in_maps(inputs):
    x = np.ascontiguousarray(np.asarray(inputs["x"], dtype=np.float32))
    target = np.asarray(inputs["target"]).astype(np.int64)
    weight = np.ascontiguousarray(
        np.asarray(inputs["weight"], dtype=np.float32))
    wtg = np.ascontiguousarray(weight[target])
    maps = []
    for c in range(NCORES):
        shard = np.ascontiguousarray(weight[c * CS:(c + 1) * CS])
        maps.append({"x": x, "w": shard, "wtg": wtg})
    return maps


def run(inputs, trace=False, **kw):
    res = run_bass_kernel_spmd(
        _get_nc(), _in_maps(inputs), core_ids=list(range(NCORES)),
        trace=trace, **kw)
    out = np.asarray(res.results[0]["out"], dtype=np.float32).reshape(())
    return out, res


def kernel(**inputs):
    out, _ = run(inputs, trace=False)
    return out
